# revision 14
# baseline (speedup 1.0000x reference)
"""Trainium2 Bass kernel for nn_Discriminator2 (bilinear discriminator scores).

Math: with hc0 = h_c[0] [N, D], W0 = W[0] [D, D]:
    v      = hc0 @ W0.T                      [N, D]   (tensor engine, bf16)
    sc1[n] = dot(h_pl[0][n], v[n]) + b       [N]      (fused DVE mult+reduce)
    sc2[s,n] = dot(hc0[sample[s,n]], v[n]) + b        (indirect-DMA gather + DVE)
    out    = [sc1 | sc2.flat | sc2.flat]     [1, N + 2*S*N]

Sharding: nodes (N) split evenly across 8 cores; hc0 replicated on every core
so gathers resolve locally; W replicated; h_pl / sample_list sharded by node.

All large operands are bf16 (host-converted): halves HBM traffic (the
bottleneck), doubles DVE throughput, and runs the PE at full rate. Dot
products accumulate in f32 (TTR accum / PSUM), keeping rel-err ~1e-3.
"""

import sys

for _p in ("/opt/trn_rl_repo",):
    if _p not in sys.path:
        sys.path.insert(0, _p)

import ml_dtypes
import numpy as np

import concourse.bass as bass
import concourse.mybir as mybir
import concourse.tile as tile
from concourse import bacc
from concourse.bass_utils import run_bass_kernel_spmd

P = 128  # partitions
BF16 = ml_dtypes.bfloat16


class Cfg:
    """Problem geometry. Full-size defaults; shrink for CoreSim validation."""

    def __init__(self, n_table=100000, nodes_per_core=12500, d=512, s=4,
                 n_cores=8, super_tile=4, gather_batch=16, use_ttr=True,
                 use_bf16=True):
        self.n_table = n_table          # rows of the gather table (full N)
        self.nodes_per_core = nodes_per_core
        self.d = d
        self.s = s
        self.n_cores = n_cores
        self.super_tile = super_tile    # node-tiles per DMA block
        self.gather_batch = gather_batch  # index columns per indirect DMA op
        self.use_ttr = use_ttr          # fused DVE mult+reduce vs mult+ACT
        self.use_bf16 = use_bf16        # bf16 operands vs f32
        self.tiles = -(-nodes_per_core // P)        # ceil
        self.npad = self.tiles * P
        self.kc = d // P                # contraction chunks


FULL = Cfg(gather_batch=1)


def build_nc(cfg: Cfg):
    D, S, KC, TILES = cfg.d, cfg.s, cfg.kc, cfg.tiles
    f32 = mybir.dt.float32
    bf16 = mybir.dt.bfloat16 if cfg.use_bf16 else f32

    nc = bacc.Bacc("TRN2", target_bir_lowering=False, debug=False,
                   num_swdge_queues=2)
    hc = nc.dram_tensor("hc", [cfg.n_table, D], bf16, kind="ExternalInput").ap()
    hcT = nc.dram_tensor("hcT", [D, cfg.npad], bf16, kind="ExternalInput").ap()
    hpl = nc.dram_tensor("hpl", [cfg.npad, D], bf16, kind="ExternalInput").ap()
    idx = nc.dram_tensor("idx", [P, TILES * S], mybir.dt.int32,
                         kind="ExternalInput").ap()
    wt = nc.dram_tensor("wt", [D, D], bf16, kind="ExternalInput").ap()
    bb = nc.dram_tensor("bb", [P, 1], f32, kind="ExternalInput").ap()
    out = nc.dram_tensor("out", [P, TILES * (S + 1)], f32,
                         kind="ExternalOutput").ap()

    with tile.TileContext(nc) as tc:
        with (
            tc.tile_pool(name="const", bufs=1) as cpool,
            tc.tile_pool(name="hcT", bufs=2) as hcT_pool,
            tc.tile_pool(name="hpl", bufs=2) as hpl_pool,
            tc.tile_pool(name="g", bufs=3) as g_pool,
            tc.tile_pool(name="v", bufs=4) as v_pool,
            tc.tile_pool(name="psum", bufs=4, space="PSUM") as psum_pool,
        ):
            # All gather indices resident: idx_sb[p, t*S+s] = sample[s, t*128+p].
            # Loaded FIRST so the gather stream (the kernel's critical path)
            # starts as early as possible.
            idx_sb = cpool.tile([P, TILES * S], mybir.dt.int32)
            nc.sync.dma_start(out=idx_sb[:], in_=idx[:])
            # W.T resident: free layout (c, d) — chunk c covers contraction
            # rows c*128..c*128+127.
            wt_sb = cpool.tile([P, KC * D], bf16)
            nc.sync.dma_start(
                out=wt_sb[:].rearrange("p (c d) -> p c d", c=KC),
                in_=wt.rearrange("(c p) d -> p c d", p=P))
            b_sb = cpool.tile([P, 1], f32)
            nc.sync.dma_start(out=b_sb[:], in_=bb[:])
            sc_acc = cpool.tile([P, TILES * (S + 1)], f32)
            dump = cpool.tile([P, D], bf16)  # discarded TTR elementwise output
            dump_f = cpool.tile([P, D], f32)  # discarded ACT output (fallback)

            qtoggle = 0
            for sti, t0 in enumerate(range(0, TILES, cfg.super_tile)):
                st = min(cfg.super_tile, TILES - t0)
                # Batched gathers per super-tile, gather_batch index columns
                # (= 128*gather_batch descriptors) per indirect DMA op:
                # g_sb[p, (j*S+s)*D:(j*S+s+1)*D] = hc[idx_sb[p, (t0+j)*S+s], :]
                g_sb = g_pool.tile([P, cfg.super_tile * S * D], bf16, tag="g")
                ncol = st * S
                gb = min(cfg.gather_batch, ncol)
                for k0 in range(0, ncol, gb):
                    kn = min(gb, ncol - k0)
                    gi = nc.gpsimd.indirect_dma_start(
                        out=g_sb[:, k0 * D:(k0 + kn) * D],
                        out_offset=None,
                        in_=hc[:],
                        in_offset=bass.IndirectOffsetOnAxis(
                            ap=idx_sb[:, t0 * S + k0:t0 * S + k0 + kn],
                            axis=0),
                    )
                    # alternate SWDGE queues so SDMA interleaves two
                    # descriptor streams (hides random-row HBM latency)
                    qtoggle ^= 1
                    if qtoggle:
                        gi.ins.queue = "qPoolDynamic1"
                # hcT block [D, st*128] -> SBUF free layout (c, n_local)
                hcT_sb = hcT_pool.tile([P, KC * cfg.super_tile * P], bf16,
                                       tag="hcT")
                nc.sync.dma_start(
                    out=hcT_sb[:, : KC * st * P].rearrange(
                        "p (c n) -> p c n", c=KC),
                    in_=hcT[:, t0 * P:(t0 + st) * P].rearrange(
                        "(c p) n -> p c n", p=P),
                )
                # hpl block: hpl_sb[p, j*D:(j+1)*D] = hpl[(t0+j)*128+p, :]
                hpl_sb = hpl_pool.tile([P, cfg.super_tile * D], bf16,
                                       tag="hpl")
                nc.sync.dma_start(
                    out=hpl_sb[:, : st * D].rearrange("p (j d) -> p j d", j=st),
                    in_=hpl[t0 * P:(t0 + st) * P, :].rearrange(
                        "(j p) d -> p j d", p=P),
                )
                for j in range(st):
                    t = t0 + j
                    # v = hc0_tile @ W.T via 4 accumulating bf16 matmuls
                    v_ps = psum_pool.tile([P, D], f32, space="PSUM", tag="v_ps")
                    for c in range(KC):
                        off = (c * st + j) * P
                        nc.tensor.matmul(
                            out=v_ps[:],
                            lhsT=hcT_sb[:, off:off + P],
                            rhs=wt_sb[:, c * D:(c + 1) * D],
                            start=(c == 0),
                            stop=(c == KC - 1),
                        )
                    # ScalarE casts v to bf16 SBUF so the DVE dot products
                    # below run at the 2x 16-bit rate.
                    v_sb = v_pool.tile([P, D], bf16, tag="v_sb")
                    nc.scalar.activation(
                        v_sb[:], v_ps[:], mybir.ActivationFunctionType.Copy)
                    # 5 dot products: fused DVE mult+reduce (TTR), or the
                    # DVE-mult + ScalarE Copy-activation accum fallback.
                    for s in range(S + 1):
                        in0 = (hpl_sb[:, j * D:(j + 1) * D] if s == 0
                               else g_sb[:, (j * S + s - 1) * D:
                                         (j * S + s) * D])
                        acc = sc_acc[:, t * (S + 1) + s:t * (S + 1) + s + 1]
                        if cfg.use_ttr:
                            nc.vector.scalar_tensor_tensor(
                                out=dump[:],
                                in0=in0,
                                scalar=1.0,
                                in1=v_sb[:],
                                op0=mybir.AluOpType.mult,
                                op1=mybir.AluOpType.mult,
                                accum_out=acc,
                            )
                        else:
                            nc.vector.tensor_mul(dump[:], in0, v_sb[:])
                            nc.scalar.activation(
                                dump_f[:], dump[:],
                                mybir.ActivationFunctionType.Copy,
                                accum_out=acc,
                            )
            nc.vector.tensor_scalar_add(sc_acc[:], sc_acc[:], b_sb[:, :1])
            nc.sync.dma_start(out=out[:], in_=sc_acc[:])
    nc.compile()
    return nc


def make_in_maps(cfg: Cfg, h_c, h_pl, sample_list, W, b):
    """Host-side sharding: full inputs -> per-core input dicts (bf16)."""
    D, S = cfg.d, cfg.s
    hc0 = np.asarray(h_c, np.float32)[0]
    hpl0 = np.asarray(h_pl, np.float32)[0]
    smp = np.asarray(sample_list)
    W0 = np.asarray(W, np.float32)[0]
    bval = float(np.asarray(b, np.float32).reshape(-1)[0])

    hdt = BF16 if cfg.use_bf16 else np.float32
    hc_bf = np.ascontiguousarray(hc0.astype(hdt))       # gather table, shared
    hcT = hc0.T                                         # [D, N] f32 view
    wt = np.ascontiguousarray(W0.T.astype(hdt))         # wt[e, d] = W[d, e]
    b_bcast = np.full((P, 1), bval, np.float32)

    in_maps = []
    for c in range(cfg.n_cores):
        lo = c * cfg.nodes_per_core
        hi = lo + cfg.nodes_per_core
        hcT_s = np.zeros((D, cfg.npad), hdt)
        hcT_s[:, : cfg.nodes_per_core] = hcT[:, lo:hi].astype(hdt)
        hpl_s = np.zeros((cfg.npad, D), hdt)
        hpl_s[: cfg.nodes_per_core] = hpl0[lo:hi].astype(hdt)
        idx_s = np.zeros((S, cfg.npad), np.int64)
        idx_s[:, : cfg.nodes_per_core] = smp[:, lo:hi]
        idx_r = np.ascontiguousarray(
            idx_s.reshape(S, cfg.tiles, P).transpose(2, 1, 0)
            .astype(np.int32).reshape(P, cfg.tiles * S))
        in_maps.append({
            "hc": hc_bf, "hcT": hcT_s, "hpl": hpl_s,
            "idx": idx_r, "wt": wt, "bb": b_bcast,
        })
    return in_maps


def assemble_output(cfg: Cfg, outs):
    """Per-core 'out' arrays [P, TILES*(S+1)] -> full logits [1, N + 2*S*N]."""
    S = cfg.s
    n = cfg.nodes_per_core * cfg.n_cores
    sc1 = np.empty((n,), np.float32)
    sc2 = np.empty((S, n), np.float32)
    for c in range(cfg.n_cores):
        o = (outs[c].reshape(P, cfg.tiles, S + 1).transpose(2, 1, 0)
             .reshape(S + 1, cfg.npad)[:, : cfg.nodes_per_core])
        lo = c * cfg.nodes_per_core
        sc1[lo:lo + cfg.nodes_per_core] = o[0]
        sc2[:, lo:lo + cfg.nodes_per_core] = o[1:]
    flat = sc2.reshape(-1)
    return np.concatenate([sc1, flat, flat])[None, :].astype(np.float32)


_NC_CACHE = {}


def _get_nc(cfg: Cfg):
    key = (cfg.n_table, cfg.nodes_per_core, cfg.d, cfg.s, cfg.super_tile,
           cfg.gather_batch, cfg.use_ttr, cfg.use_bf16)
    if key not in _NC_CACHE:
        _NC_CACHE[key] = build_nc(cfg)
    return _NC_CACHE[key]


def run_on_hw(cfg: Cfg, inputs, trace=False, trace_kwargs={}):
    nc = _get_nc(cfg)
    in_maps = make_in_maps(cfg, **inputs)
    res = run_bass_kernel_spmd(nc, in_maps, core_ids=list(range(cfg.n_cores)),
                               trace=trace, trace_kwargs=trace_kwargs)
    out = assemble_output(cfg, [r["out"] for r in res.results])
    return out, res


def kernel(h_c, h_pl, sample_list, W, b):
    inputs = dict(h_c=h_c, h_pl=h_pl, sample_list=sample_list, W=W, b=b)
    out, _ = run_on_hw(FULL, inputs, trace=False)
    return out


# revision 15
# speedup vs baseline: 1.0731x; 1.0731x over previous
"""Trainium2 Bass kernel for nn_Discriminator2 (bilinear discriminator scores).

Math: with hc0 = h_c[0] [N, D], W0 = W[0] [D, D]:
    v      = hc0 @ W0.T                      [N, D]   (tensor engine, bf16)
    sc1[n] = dot(h_pl[0][n], v[n]) + b       [N]      (fused DVE mult+reduce)
    sc2[s,n] = dot(hc0[sample[s,n]], v[n]) + b        (indirect-DMA gather + DVE)
    out    = [sc1 | sc2.flat | sc2.flat]     [1, N + 2*S*N]

Sharding: nodes (N) split evenly across 8 cores; hc0 replicated on every core
so gathers resolve locally; W replicated; h_pl / sample_list sharded by node.

All large operands are bf16 (host-converted): halves HBM traffic (the
bottleneck), doubles DVE throughput, and runs the PE at full rate. Dot
products accumulate in f32 (TTR accum / PSUM), keeping rel-err ~1e-3.
"""

import sys

for _p in ("/opt/trn_rl_repo",):
    if _p not in sys.path:
        sys.path.insert(0, _p)

import ml_dtypes
import numpy as np

import concourse.bass as bass
import concourse.mybir as mybir
import concourse.tile as tile
from concourse import bacc
from concourse.bass_utils import run_bass_kernel_spmd

P = 128  # partitions
BF16 = ml_dtypes.bfloat16


class Cfg:
    """Problem geometry. Full-size defaults; shrink for CoreSim validation."""

    def __init__(self, n_table=100000, nodes_per_core=12500, d=512, s=4,
                 n_cores=8, super_tile=4, gather_batch=16, use_ttr=True,
                 use_bf16=True):
        self.n_table = n_table          # rows of the gather table (full N)
        self.nodes_per_core = nodes_per_core
        self.d = d
        self.s = s
        self.n_cores = n_cores
        self.super_tile = super_tile    # node-tiles per DMA block
        self.gather_batch = gather_batch  # index columns per indirect DMA op
        self.use_ttr = use_ttr          # fused DVE mult+reduce vs mult+ACT
        self.use_bf16 = use_bf16        # bf16 operands vs f32
        self.tiles = -(-nodes_per_core // P)        # ceil
        self.npad = self.tiles * P
        self.kc = d // P                # contraction chunks


FULL = Cfg(gather_batch=1)


def build_nc(cfg: Cfg):
    D, S, KC, TILES = cfg.d, cfg.s, cfg.kc, cfg.tiles
    f32 = mybir.dt.float32
    bf16 = mybir.dt.bfloat16 if cfg.use_bf16 else f32

    nc = bacc.Bacc("TRN2", target_bir_lowering=False, debug=False,
                   num_swdge_queues=4)
    hc = nc.dram_tensor("hc", [cfg.n_table, D], bf16, kind="ExternalInput").ap()
    hcT = nc.dram_tensor("hcT", [D, cfg.npad], bf16, kind="ExternalInput").ap()
    hpl = nc.dram_tensor("hpl", [cfg.npad, D], bf16, kind="ExternalInput").ap()
    idx = nc.dram_tensor("idx", [P, TILES * S], mybir.dt.int32,
                         kind="ExternalInput").ap()
    wt = nc.dram_tensor("wt", [D, D], bf16, kind="ExternalInput").ap()
    bb = nc.dram_tensor("bb", [P, 1], f32, kind="ExternalInput").ap()
    out = nc.dram_tensor("out", [P, TILES * (S + 1)], f32,
                         kind="ExternalOutput").ap()

    with tile.TileContext(nc) as tc:
        with (
            tc.tile_pool(name="const", bufs=1) as cpool,
            tc.tile_pool(name="hcT", bufs=3) as hcT_pool,
            tc.tile_pool(name="hpl", bufs=3) as hpl_pool,
            tc.tile_pool(name="g", bufs=4) as g_pool,
            tc.tile_pool(name="psum", bufs=4, space="PSUM") as psum_pool,
        ):
            # All gather indices resident: idx_sb[p, t*S+s] = sample[s, t*128+p].
            # Loaded FIRST so the gather stream (the kernel's critical path)
            # starts as early as possible.
            idx_sb = cpool.tile([P, TILES * S], mybir.dt.int32)
            nc.sync.dma_start(out=idx_sb[:], in_=idx[:])
            # W.T resident: free layout (c, d) — chunk c covers contraction
            # rows c*128..c*128+127.
            wt_sb = cpool.tile([P, KC * D], bf16)
            nc.sync.dma_start(
                out=wt_sb[:].rearrange("p (c d) -> p c d", c=KC),
                in_=wt.rearrange("(c p) d -> p c d", p=P))
            b_sb = cpool.tile([P, 1], f32)
            nc.sync.dma_start(out=b_sb[:], in_=bb[:])
            sc_acc = cpool.tile([P, TILES * (S + 1)], f32)
            dump = cpool.tile([P, D], bf16)  # discarded TTR elementwise output
            dump_f = cpool.tile([P, D], f32)  # discarded ACT output (fallback)

            qtoggle = 0
            for sti, t0 in enumerate(range(0, TILES, cfg.super_tile)):
                st = min(cfg.super_tile, TILES - t0)
                # Batched gathers per super-tile, gather_batch index columns
                # (= 128*gather_batch descriptors) per indirect DMA op:
                # g_sb[p, (j*S+s)*D:(j*S+s+1)*D] = hc[idx_sb[p, (t0+j)*S+s], :]
                g_sb = g_pool.tile([P, cfg.super_tile * S * D], bf16, tag="g")
                ncol = st * S
                gb = min(cfg.gather_batch, ncol)
                for k0 in range(0, ncol, gb):
                    kn = min(gb, ncol - k0)
                    gi = nc.gpsimd.indirect_dma_start(
                        out=g_sb[:, k0 * D:(k0 + kn) * D],
                        out_offset=None,
                        in_=hc[:],
                        in_offset=bass.IndirectOffsetOnAxis(
                            ap=idx_sb[:, t0 * S + k0:t0 * S + k0 + kn],
                            axis=0),
                    )
                    # round-robin SWDGE queues so SDMA interleaves several
                    # descriptor streams (hides random-row HBM latency)
                    if qtoggle % 4:
                        gi.ins.queue = f"qPoolDynamic{qtoggle % 4}"
                    qtoggle += 1
                # hcT block [D, st*128] -> SBUF free layout (c, n_local)
                hcT_sb = hcT_pool.tile([P, KC * cfg.super_tile * P], bf16,
                                       tag="hcT")
                nc.sync.dma_start(
                    out=hcT_sb[:, : KC * st * P].rearrange(
                        "p (c n) -> p c n", c=KC),
                    in_=hcT[:, t0 * P:(t0 + st) * P].rearrange(
                        "(c p) n -> p c n", p=P),
                )
                # hpl block: hpl_sb[p, j*D:(j+1)*D] = hpl[(t0+j)*128+p, :]
                hpl_sb = hpl_pool.tile([P, cfg.super_tile * D], bf16,
                                       tag="hpl")
                nc.sync.dma_start(
                    out=hpl_sb[:, : st * D].rearrange("p (j d) -> p j d", j=st),
                    in_=hpl[t0 * P:(t0 + st) * P, :].rearrange(
                        "(j p) d -> p j d", p=P),
                )
                for j in range(st):
                    t = t0 + j
                    # v = hc0_tile @ W.T via 4 accumulating bf16 matmuls
                    v_ps = psum_pool.tile([P, D], f32, space="PSUM", tag="v_ps")
                    for c in range(KC):
                        off = (c * st + j) * P
                        nc.tensor.matmul(
                            out=v_ps[:],
                            lhsT=hcT_sb[:, off:off + P],
                            rhs=wt_sb[:, c * D:(c + 1) * D],
                            start=(c == 0),
                            stop=(c == KC - 1),
                        )
                    # 5 dot products: fused DVE mult+reduce (STT reads v
                    # straight from PSUM — STT runs at 1x rate regardless of
                    # dtype, so a bf16 staging copy buys nothing), or the
                    # DVE-mult + ScalarE Copy-activation accum fallback.
                    for s in range(S + 1):
                        in0 = (hpl_sb[:, j * D:(j + 1) * D] if s == 0
                               else g_sb[:, (j * S + s - 1) * D:
                                         (j * S + s) * D])
                        acc = sc_acc[:, t * (S + 1) + s:t * (S + 1) + s + 1]
                        if cfg.use_ttr:
                            nc.vector.scalar_tensor_tensor(
                                out=dump[:],
                                in0=in0,
                                scalar=1.0,
                                in1=v_ps[:],
                                op0=mybir.AluOpType.mult,
                                op1=mybir.AluOpType.mult,
                                accum_out=acc,
                            )
                        else:
                            nc.vector.tensor_mul(dump[:], in0, v_ps[:])
                            nc.scalar.activation(
                                dump_f[:], dump[:],
                                mybir.ActivationFunctionType.Copy,
                                accum_out=acc,
                            )
            nc.vector.tensor_scalar_add(sc_acc[:], sc_acc[:], b_sb[:, :1])
            nc.sync.dma_start(out=out[:], in_=sc_acc[:])
    nc.compile()
    return nc


def make_in_maps(cfg: Cfg, h_c, h_pl, sample_list, W, b):
    """Host-side sharding: full inputs -> per-core input dicts (bf16)."""
    D, S = cfg.d, cfg.s
    hc0 = np.asarray(h_c, np.float32)[0]
    hpl0 = np.asarray(h_pl, np.float32)[0]
    smp = np.asarray(sample_list)
    W0 = np.asarray(W, np.float32)[0]
    bval = float(np.asarray(b, np.float32).reshape(-1)[0])

    hdt = BF16 if cfg.use_bf16 else np.float32
    hc_bf = np.ascontiguousarray(hc0.astype(hdt))       # gather table, shared
    hcT = hc0.T                                         # [D, N] f32 view
    wt = np.ascontiguousarray(W0.T.astype(hdt))         # wt[e, d] = W[d, e]
    b_bcast = np.full((P, 1), bval, np.float32)

    in_maps = []
    for c in range(cfg.n_cores):
        lo = c * cfg.nodes_per_core
        hi = lo + cfg.nodes_per_core
        hcT_s = np.zeros((D, cfg.npad), hdt)
        hcT_s[:, : cfg.nodes_per_core] = hcT[:, lo:hi].astype(hdt)
        hpl_s = np.zeros((cfg.npad, D), hdt)
        hpl_s[: cfg.nodes_per_core] = hpl0[lo:hi].astype(hdt)
        idx_s = np.zeros((S, cfg.npad), np.int64)
        idx_s[:, : cfg.nodes_per_core] = smp[:, lo:hi]
        idx_r = np.ascontiguousarray(
            idx_s.reshape(S, cfg.tiles, P).transpose(2, 1, 0)
            .astype(np.int32).reshape(P, cfg.tiles * S))
        in_maps.append({
            "hc": hc_bf, "hcT": hcT_s, "hpl": hpl_s,
            "idx": idx_r, "wt": wt, "bb": b_bcast,
        })
    return in_maps


def assemble_output(cfg: Cfg, outs):
    """Per-core 'out' arrays [P, TILES*(S+1)] -> full logits [1, N + 2*S*N]."""
    S = cfg.s
    n = cfg.nodes_per_core * cfg.n_cores
    sc1 = np.empty((n,), np.float32)
    sc2 = np.empty((S, n), np.float32)
    for c in range(cfg.n_cores):
        o = (outs[c].reshape(P, cfg.tiles, S + 1).transpose(2, 1, 0)
             .reshape(S + 1, cfg.npad)[:, : cfg.nodes_per_core])
        lo = c * cfg.nodes_per_core
        sc1[lo:lo + cfg.nodes_per_core] = o[0]
        sc2[:, lo:lo + cfg.nodes_per_core] = o[1:]
    flat = sc2.reshape(-1)
    return np.concatenate([sc1, flat, flat])[None, :].astype(np.float32)


_NC_CACHE = {}


def _get_nc(cfg: Cfg):
    key = (cfg.n_table, cfg.nodes_per_core, cfg.d, cfg.s, cfg.super_tile,
           cfg.gather_batch, cfg.use_ttr, cfg.use_bf16)
    if key not in _NC_CACHE:
        _NC_CACHE[key] = build_nc(cfg)
    return _NC_CACHE[key]


def run_on_hw(cfg: Cfg, inputs, trace=False, trace_kwargs={}):
    nc = _get_nc(cfg)
    in_maps = make_in_maps(cfg, **inputs)
    res = run_bass_kernel_spmd(nc, in_maps, core_ids=list(range(cfg.n_cores)),
                               trace=trace, trace_kwargs=trace_kwargs)
    out = assemble_output(cfg, [r["out"] for r in res.results])
    return out, res


def kernel(h_c, h_pl, sample_list, W, b):
    inputs = dict(h_c=h_c, h_pl=h_pl, sample_list=sample_list, W=W, b=b)
    out, _ = run_on_hw(FULL, inputs, trace=False)
    return out
